# revision 20
# baseline (speedup 1.0000x reference)
"""Bahdanau-style additive attention on 8 Trainium2 NeuronCores.

Math (per batch row b):
    q_proj = query @ Wa_w.T + Wa_b                      # (H,)
    k_proj = keys @ Ua_w.T + Ua_b                       # (L, H)
    hidden = tanh(q_proj + k_proj)                      # (L, H)
    scores = hidden @ Va_w[0] + Va_b[0]                 # (L,)
    attn   = softmax(mask ? scores : -inf)              # (L,)
    context = attn @ keys                               # (H,)
    return (context, attn)

Sharding: data-parallel over batch B=64 -> 8 batches per core. Weights are
tiny and replicated.

Device design:
  - keys are fed pre-transposed per batch as keysT[h, l] (h on partitions,
    4 chunks of 128) so the big GEMM kpT[g, l] = Ua[g, :] @ keysT[:, l]
    contracts over h on the partition dim. bf16 by default.
  - kpT comes out of PSUM as [g(128), l(512)] tiles; ScalarE applies tanh
    with per-partition bias qp[g] (q_proj + Wa_b + Ua_b folded on host).
  - scores = Va . hidden via M=1 matmuls accumulating over the 4 g-chunks.
  - softmax on partition 0: per-chunk Exp on ScalarE reads the scores PSUM
    directly (bias=Va_b, accum_out gives the denominator for free); no max
    subtraction needed (|scores| <= sum|Va| + |Va_b| ~ 23 is safe in fp32).
  - The mask is all ones per the problem spec; a masked fallback build is
    compiled lazily if a non-trivial mask ever shows up.
  - context: attn row -> DRAM -> broadcast-load (with fp32->bf16 cast) to
    128 partitions; VectorE multiply + free-axis reduce against the
    SBUF-resident keysT tiles.

q_proj (17 MFLOP of the 137 GFLOP total) is computed on the host and folded
into the tanh bias.
"""

import os

import numpy as np
import ml_dtypes

import concourse.bass as bass
import concourse.mybir as mybir
import concourse.tile as tile
from concourse import bacc
from concourse.bass_utils import run_bass_kernel_spmd

B, L, H = 64, 4096, 512
NCORES = 8
BLOC = B // NCORES  # 8 batch rows per core
P = 128
HC = H // P         # 4 h-chunks
GC = H // P         # 4 g-chunks
LCH = 512           # matmul moving free dim
NLC = L // LCH      # 8 l-chunks

# "bf16": host casts keys/Ua to bf16 (halves HBM traffic, fastest).
# "f32r": keys/Ua stay fp32 in HBM, matmul in fp32r (single pass, more bits).
KEYS_DTYPE = os.environ.get("KERNEL_KEYS_DTYPE", "fp16")
# Repeat the whole 8-batch computation REPS times inside the NEFF (for
# wall-clock timing: diff between REPS=r and REPS=1 isolates HW time).
REPS = int(os.environ.get("KERNEL_REPS", "1"))

TRACE = False
LAST_RESULT = None

_BUILD_CACHE = {}


def _build(keys_dtype: str, use_mask: bool, reps: int = 1) -> bass.Bass:
    nc = bacc.Bacc(None, target_bir_lowering=False)
    f32 = mybir.dt.float32
    bf16 = mybir.dt.bfloat16
    fp16 = mybir.dt.float16
    if keys_dtype == "bf16":
        kdt = bf16
        mmdt = bf16
        keys_bufs = 9
    elif keys_dtype == "fp16":
        kdt = fp16
        mmdt = fp16
        keys_bufs = 9
    else:
        kdt = mybir.dt.float32r
        mmdt = mybir.dt.float32r
        keys_bufs = 6
    AF = mybir.ActivationFunctionType
    ALU = mybir.AluOpType
    h16 = kdt if kdt in (bf16, fp16) else fp16

    keysT = nc.dram_tensor("keysT", (BLOC, HC, P, L), kdt, kind="ExternalInput")
    ua = nc.dram_tensor("ua", (P, HC, H), kdt, kind="ExternalInput")
    qp = nc.dram_tensor("qp", (P, GC, BLOC), f32, kind="ExternalInput")
    va = nc.dram_tensor("va", (P, GC), h16, kind="ExternalInput")
    vab = nc.dram_tensor("vab", (1, 1), f32, kind="ExternalInput")
    if use_mask:
        madd = nc.dram_tensor("madd", (BLOC, L), f32, kind="ExternalInput")
    ctxT = nc.dram_tensor("ctxT", (BLOC, P, HC), f32, kind="ExternalOutput")
    attn = nc.dram_tensor("attn", (BLOC, L), f32, kind="ExternalOutput")
    deno = nc.dram_tensor("deno", (BLOC, 1), f32, kind="ExternalOutput")

    with tile.TileContext(nc) as tc:
        with (
            tc.tile_pool(name="consts", bufs=1) as consts,
            tc.tile_pool(name="keys", bufs=keys_bufs) as keys_pool,
            tc.tile_pool(name="hid", bufs=6) as hid_pool,
            tc.tile_pool(name="small", bufs=2) as small,
            tc.tile_pool(name="tmp", bufs=4) as tmp_pool,
            tc.tile_pool(name="bc", bufs=4) as bc_pool,
            tc.tile_pool(name="ctx", bufs=2) as ctx_pool,
            tc.tile_pool(name="pkp", bufs=4, space="PSUM") as psum_kp,
            tc.tile_pool(name="psc", bufs=3, space="PSUM") as psum_sc,
            tc.tile_pool(name="dram", bufs=4, space="DRAM") as dram_pool,
        ):
            ua_sb = consts.tile([P, HC, H], kdt)
            nc.sync.dma_start(ua_sb, ua[:, :, :])
            qp_sb = consts.tile([P, GC, BLOC], f32)
            nc.sync.dma_start(qp_sb, qp[:, :, :])
            va_sb = consts.tile([P, GC], h16)
            nc.sync.dma_start(va_sb, va[:, :])
            vab_sb = consts.tile([1, 1], f32)
            nc.sync.dma_start(vab_sb, vab[:, :])

            # Priming matmuls: absorb the const-DMA waits on the PE queue once,
            # so steady-state matmuls carry fewer sync waits (walrus limits
            # wait commands per instruction, especially for fp32r).
            with tc.tile_pool(name="prime", bufs=1, space="PSUM") as prime_pool:
                pr = prime_pool.tile([1, 1], f32, tag="pr")
                nc.tensor.matmul(
                    pr,
                    lhsT=ua_sb[:, 0, 0:1].bitcast(mmdt),
                    rhs=ua_sb[:, 0, 0:1].bitcast(mmdt),
                    start=True,
                    stop=True,
                )
                pr2 = prime_pool.tile([1, 1], f32, tag="pr")
                nc.tensor.matmul(
                    pr2, lhsT=va_sb[:, 0:1], rhs=va_sb[:, 0:1], start=True, stop=True
                )

            for rep in range(reps):
              for b in range(BLOC):
                # Load this batch's keysT: 4 tiles of [128 h, 4096 l]
                kt = []
                for hc in range(HC):
                    t = keys_pool.tile([P, L], kdt, tag="keysT")
                    nc.sync.dma_start(t, keysT[b, hc, :, :])
                    kt.append(t)

                # Unnormalized softmax weights w = exp(scores + Va_b); both
                # outputs are normalized by den on the host.
                w_sb = small.tile([1, L], f32, tag="w")
                den8 = small.tile([1, NLC], f32, tag="den8")
                # Partial context sums per (h-chunk, l-chunk)
                ctx8 = ctx_pool.tile([P, HC, NLC], f32, tag="ctx8")
                if use_mask:
                    madd_sb = small.tile([1, L], f32, tag="madd")
                    nc.sync.dma_start(madd_sb, madd[b : b + 1, :])
                    scores_sb = small.tile([1, L], f32, tag="scores")

                for lc in range(NLC):
                    lsl = slice(lc * LCH, (lc + 1) * LCH)
                    ps_s = psum_sc.tile([1, LCH], f32, tag="ps_s")
                    for gc in range(GC):
                        ps_kp = psum_kp.tile([P, LCH], f32, tag="ps_kp")
                        for hc in range(HC):
                            nc.tensor.matmul(
                                ps_kp,
                                lhsT=ua_sb[:, hc, gc * P : (gc + 1) * P].bitcast(mmdt),
                                rhs=kt[hc][:, lsl].bitcast(mmdt),
                                start=(hc == 0),
                                stop=(hc == HC - 1),
                            )
                        hid = hid_pool.tile([P, LCH], h16, tag="hid")
                        nc.scalar.activation(
                            hid,
                            ps_kp,
                            AF.Tanh,
                            bias=qp_sb[:, gc, b : b + 1],
                            scale=1.0,
                        )
                        nc.tensor.matmul(
                            ps_s,
                            lhsT=va_sb[:, gc : gc + 1],
                            rhs=hid,
                            start=(gc == 0),
                            stop=(gc == GC - 1),
                        )
                    if use_mask:
                        # scores = ps_s + Va_b; mask + exp happen after the loop
                        nc.scalar.activation(
                            scores_sb[:, lsl], ps_s, AF.Identity,
                            bias=vab_sb[:, :], scale=1.0,
                        )
                    else:
                        # w[lsl] = exp(ps_s + Va_b); den8[lc] = sum(w[lsl])
                        nc.scalar.activation(
                            w_sb[:, lsl],
                            ps_s,
                            AF.Exp,
                            bias=vab_sb[:, :],
                            scale=1.0,
                            accum_out=den8[:, lc : lc + 1],
                        )

                    if not use_mask:
                        # Per-chunk context: broadcast this w chunk to 128
                        # partitions (bf16) and accumulate partial sums, fully
                        # overlapped with later chunks' matmuls.
                        scr = dram_pool.tile([1, LCH], f32, tag="scr")
                        nc.sync.dma_start(scr, w_sb[:, lsl])
                        wbc = bc_pool.tile([P, LCH], h16, tag="wbc")
                        nc.gpsimd.dma_start(wbc, scr.to_broadcast((P, LCH)))
                        for hc in range(HC):
                            tmp = tmp_pool.tile([P, LCH], h16, tag="tmp")
                            if kdt in (bf16, fp16):
                                nc.vector.tensor_mul(tmp, kt[hc][:, lsl], wbc)
                            else:
                                nc.vector.tensor_mul(
                                    tmp, kt[hc][:, lsl].bitcast(f32), wbc
                                )
                            nc.vector.reduce_sum(
                                ctx8[:, hc, lc : lc + 1],
                                tmp,
                                axis=mybir.AxisListType.X,
                            )

                if use_mask:
                    nc.vector.tensor_add(scores_sb, scores_sb, madd_sb)
                    den8 = small.tile([1, 1], f32, tag="den1")
                    nc.scalar.activation(w_sb, scores_sb, AF.Exp, accum_out=den8)
                    for lc in range(NLC):
                        lsl = slice(lc * LCH, (lc + 1) * LCH)
                        scr = dram_pool.tile([1, LCH], f32, tag="scr")
                        nc.sync.dma_start(scr, w_sb[:, lsl])
                        wbc = bc_pool.tile([P, LCH], h16, tag="wbc")
                        nc.gpsimd.dma_start(wbc, scr.to_broadcast((P, LCH)))
                        for hc in range(HC):
                            tmp = tmp_pool.tile([P, LCH], h16, tag="tmp")
                            if kdt in (bf16, fp16):
                                nc.vector.tensor_mul(tmp, kt[hc][:, lsl], wbc)
                            else:
                                nc.vector.tensor_mul(
                                    tmp, kt[hc][:, lsl].bitcast(f32), wbc
                                )
                            nc.vector.reduce_sum(
                                ctx8[:, hc, lc : lc + 1],
                                tmp,
                                axis=mybir.AxisListType.X,
                            )

                den = small.tile([1, 1], f32, tag="den")
                nc.vector.reduce_sum(den, den8, axis=mybir.AxisListType.X)
                nc.sync.dma_start(deno[b : b + 1, :], den)
                nc.sync.dma_start(attn[b : b + 1, :], w_sb)

                # ctx_raw[h] = sum over the NLC partials
                ctx = ctx_pool.tile([P, HC], f32, tag="ctx")
                nc.vector.reduce_sum(ctx, ctx8, axis=mybir.AxisListType.X)
                nc.sync.dma_start(ctxT[b, :, :], ctx)

    nc.compile()
    return nc


def _get_nc(keys_dtype: str, use_mask: bool = False, reps: int | None = None) -> bass.Bass:
    if reps is None:
        reps = REPS
    key = (keys_dtype, use_mask, reps)
    if key not in _BUILD_CACHE:
        _BUILD_CACHE[key] = _build(keys_dtype, use_mask, reps)
    return _BUILD_CACHE[key]


def build_in_maps(inputs, use_mask: bool = False):
    query = np.asarray(inputs["query"], dtype=np.float32)
    keys = np.asarray(inputs["keys"], dtype=np.float32)
    mask = np.asarray(inputs["mask"])
    Wa_w = np.asarray(inputs["Wa_w"], dtype=np.float32)
    Wa_b = np.asarray(inputs["Wa_b"], dtype=np.float32)
    Ua_w = np.asarray(inputs["Ua_w"], dtype=np.float32)
    Ua_b = np.asarray(inputs["Ua_b"], dtype=np.float32)
    Va_w = np.asarray(inputs["Va_w"], dtype=np.float32)
    Va_b = np.asarray(inputs["Va_b"], dtype=np.float32)

    if KEYS_DTYPE == "bf16":
        kdt_np = ml_dtypes.bfloat16
    elif KEYS_DTYPE == "fp16":
        kdt_np = np.float16
    else:
        kdt_np = np.float32
    h16_np = kdt_np if KEYS_DTYPE in ("bf16", "fp16") else np.float16

    # Host-side prep (cheap relative to the 137 GFLOP device GEMM):
    # fold q_proj and both biases into the per-(g, b) tanh bias.
    qp_all = (query @ Wa_w.T + Wa_b + Ua_b).astype(np.float32)  # (B, H)

    # keysT[b, hc, hp, l] = keys[b, l, hc*128+hp]
    keysT_all = np.ascontiguousarray(
        keys.reshape(B, L, HC, P).transpose(0, 2, 3, 1)
    ).astype(kdt_np, copy=False)

    # ua[hp, hc, g] = Ua_w[g, hc*128+hp]
    ua_np = np.ascontiguousarray(
        Ua_w.T.reshape(HC, P, H).transpose(1, 0, 2)
    ).astype(kdt_np, copy=False)

    va_np = np.ascontiguousarray(Va_w[0].reshape(GC, P).T).astype(h16_np)
    # Shift the exp by a constant so the fp16 broadcast of the unnormalized
    # weights cannot overflow: |scores| <= sum|Va| + |Va_b|, so
    # exp(scores + Va_b - C) <= e^10.5 << fp16 max. Softmax is shift-invariant
    # (both outputs are normalized by den downstream).
    shift = float(np.abs(Va_w).sum() + np.abs(Va_b).sum()) - 10.5
    shift = max(shift, 0.0)
    vab_np = (Va_b.reshape(1, 1) - shift).astype(np.float32)

    in_maps = []
    for c in range(NCORES):
        b0 = c * BLOC
        qp_np = np.ascontiguousarray(
            qp_all[b0 : b0 + BLOC].T.reshape(GC, P, BLOC).transpose(1, 0, 2)
        )
        m = {
            "keysT": keysT_all[b0 : b0 + BLOC],
            "ua": ua_np,
            "qp": qp_np,
            "va": va_np,
            "vab": vab_np,
        }
        if use_mask:
            madd_all = np.where(mask, 0.0, -1e30).astype(np.float32)
            m["madd"] = madd_all[b0 : b0 + BLOC]
        in_maps.append(m)
    return in_maps


def kernel(query, keys, mask, Wa_w, Wa_b, Ua_w, Ua_b, Va_w, Va_b):
    global LAST_RESULT
    inputs = dict(
        query=query, keys=keys, mask=mask, Wa_w=Wa_w, Wa_b=Wa_b,
        Ua_w=Ua_w, Ua_b=Ua_b, Va_w=Va_w, Va_b=Va_b,
    )
    use_mask = not bool(np.asarray(mask).all())
    in_maps = build_in_maps(inputs, use_mask=use_mask)
    nc = _get_nc(KEYS_DTYPE, use_mask)
    res = run_bass_kernel_spmd(
        nc, in_maps, core_ids=list(range(NCORES)), trace=TRACE
    )
    LAST_RESULT = res

    context = np.empty((B, H), dtype=np.float32)
    attn_w = np.empty((B, L), dtype=np.float32)
    for c in range(NCORES):
        b0 = c * BLOC
        out = res.results[c]
        den = out["deno"].astype(np.float64)  # (BLOC, 1)
        # ctxT[b, hp, hc] -> context[b, hc*128+hp]; normalize by softmax denom
        ctx_raw = out["ctxT"].transpose(0, 2, 1).reshape(BLOC, H)
        context[b0 : b0 + BLOC] = ctx_raw / den
        attn_w[b0 : b0 + BLOC] = out["attn"] / den
    return (context, attn_w)
